# revision 16
# baseline (speedup 1.0000x reference)
"""GCN message-passing model on 8 Trainium2 NeuronCores.

Strategy (graph/data parallel, dst-sharded edges, replicated node table):
  - Nodes are sharded 6250/core (padded to 6272 = 49*128 local slots).
  - Each core owns all edges whose dst lands in its shard; edges are
    sorted by dst tile and gathered from a replicated [50176,128] bf16
    node table in DRAM via hardware dma_gather (int16 indices force a
    lo/hi split of the table).
  - segment_sum is computed as one-hot matmuls into PSUM (bf16 msg x
    bf16 one-hot, fp32 accumulate), producing the aggregate transposed
    so the shared-weight GCN matmul needs no extra transposes.
  - After each round every core rewrites its shard of h/deg (bf16) and
    an AllGather refreshes the replicated table.
  - The unused edge_feat branch of the reference is skipped entirely.
  - Final stack-mean is an AllReduce of per-core column sums; every
    core redundantly computes the tiny head (Wpred/Wcls/softmax).
"""
import sys

for _p in ("/opt/trn_rl_repo", "/opt/trn_rl_repo/concourse"):
    if _p not in sys.path:
        sys.path.insert(0, _p)

import os
import numpy as np
import ml_dtypes

import concourse.bass as bass
import concourse.bacc as bacc
import concourse.mybir as mybir
import concourse.tile as tile
from concourse import bass_utils

P = 8                  # cores
N_REAL = 50000
NS_REAL = 6250         # real nodes per core
NS = 6272              # padded nodes per core (49 * 128)
NT = 49                # node tiles per core
NPAD = NS * P          # 50176
HALF = NPAD // 2       # 25088  (int16 gather index limit is 32767)
H = 128
NF = 256
E = 800000
GROUP = 4              # dst tiles per gather chunk
NEG = 0.01

f32 = mybir.dt.float32
bf16 = mybir.dt.bfloat16
i16 = mybir.dt.int16
AX = mybir.AxisListType.X
OP = mybir.AluOpType


def _wrap_idxs(idxs):
    """[E_PAD] int -> [128, E_PAD//16] int16, 16-partition wrap replicated x8."""
    n = idxs.shape[0]
    w = idxs.reshape(n // 16, 16).T.astype(np.int16)
    return np.ascontiguousarray(np.tile(w, (8, 1)))


def _preprocess(node_feat, degree, src, dst, Wn, bn, We, be, Wgcn, bgcn,
                Wpred, bpred, Wcls, bcls, base_data, edge_feat):
    """Host-side sharding. Returns (in_maps, meta)."""
    src = np.asarray(src).astype(np.int64)
    dst = np.asarray(dst).astype(np.int64)
    node_feat = np.asarray(node_feat, dtype=np.float32)
    degree = np.asarray(degree, dtype=np.float32).reshape(-1)

    core_of = dst // NS_REAL
    dl = dst - core_of * NS_REAL          # dst local id 0..6249
    tile_of = dl // 128
    dcol = dl % 128
    sp = (src // NS_REAL) * NS + (src % NS_REAL)   # padded global src id
    half = (sp >= HALF).astype(np.int64)
    sidx = sp - half * HALF               # local index within half, < 25088

    # exact in-degree per node (for the fusion-round bias correction)
    indeg = np.zeros(N_REAL, np.float32)
    np.add.at(indeg, dst, 1.0)

    # bucket edges per (core, tile, half); record counts to get cross-core maxes
    order = np.lexsort((sidx, half, tile_of, core_of))
    core_s, tile_s, half_s, sidx_s, dcol_s = (
        core_of[order], tile_of[order], half[order], sidx[order], dcol[order])
    key = (core_s * NT + tile_s) * 2 + half_s
    counts = np.bincount(key, minlength=P * NT * 2).reshape(P, NT, 2)
    nblk = (counts + 127) // 128          # blocks per (core,tile,half)
    nlo = nblk[:, :, 0].max(axis=0)       # cross-core max, per tile
    nhi = nblk[:, :, 1].max(axis=0)
    NB = int(nlo.sum() + nhi.sum())
    E_PAD = NB * 128

    # group structure: for each group of GROUP tiles, lo blocks of its tiles
    # then hi blocks. Per-tile block positions are compile-time constants.
    groups = [list(range(g, min(g + GROUP, NT))) for g in range(0, NT, GROUP)]
    meta = dict(nlo=nlo.tolist(), nhi=nhi.tolist(), groups=groups, NB=NB,
                E_PAD=E_PAD)

    # per-core edge arrays in final order
    starts = np.zeros(P * NT * 2 + 1, np.int64)
    np.cumsum(counts.reshape(-1), out=starts[1:])
    in_maps = []
    for c in range(P):
        idx_arr = np.zeros(E_PAD, np.int64)
        dcol_arr = np.full(E_PAD, 128.0, np.float32)   # 128 => dead one-hot row
        pos = 0
        for g in groups:
            for hf in (0, 1):
                for t in g:
                    k = (c * NT + t) * 2 + hf
                    s, e = starts[k], starts[k + 1]
                    nb = int(nlo[t] if hf == 0 else nhi[t])
                    idx_arr[pos:pos + (e - s)] = sidx_s[s:e]
                    dcol_arr[pos:pos + (e - s)] = dcol_s[s:e]
                    pos += nb * 128
        assert pos == E_PAD

        nf_sh = np.zeros((NS, NF), np.float32)
        nf_sh[:NS_REAL] = node_feat[c * NS_REAL:(c + 1) * NS_REAL]
        deg_sh = np.ones(NS, np.float32)
        deg_sh[:NS_REAL] = degree[c * NS_REAL:(c + 1) * NS_REAL]
        rdeg = (1.0 / deg_sh).astype(np.float32)
        ind_sh = np.zeros(NS, np.float32)
        ind_sh[:NS_REAL] = indeg[c * NS_REAL:(c + 1) * NS_REAL]

        in_maps.append(dict(
            nf=nf_sh,
            idxs=_wrap_idxs(idx_arr),
            dstl=np.ascontiguousarray(
                dcol_arr.reshape(NB, 128).T).astype(ml_dtypes.bfloat16),
            rdeg=np.ascontiguousarray(rdeg.reshape(NT, 128).T),   # [128, NT]
            indeg=ind_sh.reshape(1, NS),                          # [1, NS]
            Wn=np.asarray(Wn, np.float32).reshape(2, 128, H).transpose(1, 0, 2).copy(),
            Wgcn=np.asarray(Wgcn, np.float32),
            Wpred=np.asarray(Wpred, np.float32).reshape(4, 128, H).transpose(1, 0, 2).copy(),
            Wcls=np.asarray(Wcls, np.float32),
            bn=np.asarray(bn, np.float32).reshape(1, H),
            bgcn=np.asarray(bgcn, np.float32).reshape(1, H),
            bpred=np.asarray(bpred, np.float32).reshape(1, H),
            bcls=np.asarray(bcls, np.float32).reshape(1, 2),
            iota=np.tile(np.arange(128, dtype=np.float32),
                         (128, 1)).astype(ml_dtypes.bfloat16),
            ident=np.eye(128, dtype=np.float32),
            ones_r=np.ones((1, 128), np.float32),
            ones_c=np.ones((128, 1), np.float32).astype(ml_dtypes.bfloat16),
        ))
    return in_maps, meta


ABLATE = os.environ.get("GCN_ABLATE", "")


def _build(meta):
    no_cc = "cc" in ABLATE
    no_gather = "gather" in ABLATE
    no_oh = "oh" in ABLATE
    no_mm = "mm" in ABLATE
    nlo, nhi, groups = meta["nlo"], meta["nhi"], meta["groups"]
    NB, E_PAD = meta["NB"], meta["E_PAD"]

    nc = bacc.Bacc("TRN2", target_bir_lowering=False, debug=False,
                   num_devices=P, num_swdge_queues=4)

    # ---- I/O ----
    nf_d = nc.dram_tensor("nf", [NS, NF], f32, kind="ExternalInput")
    idxs_d = nc.dram_tensor("idxs", [128, E_PAD // 16], i16, kind="ExternalInput")
    dstl_d = nc.dram_tensor("dstl", [128, NB], bf16, kind="ExternalInput")
    rdeg_d = nc.dram_tensor("rdeg", [128, NT], f32, kind="ExternalInput")
    indeg_d = nc.dram_tensor("indeg", [1, NS], f32, kind="ExternalInput")
    wn_d = nc.dram_tensor("Wn", [128, 2, H], f32, kind="ExternalInput")
    wg_d = nc.dram_tensor("Wgcn", [128, H], f32, kind="ExternalInput")
    wp_d = nc.dram_tensor("Wpred", [128, 4, H], f32, kind="ExternalInput")
    wc_d = nc.dram_tensor("Wcls", [128, 2], f32, kind="ExternalInput")
    bn_d = nc.dram_tensor("bn", [1, H], f32, kind="ExternalInput")
    bg_d = nc.dram_tensor("bgcn", [1, H], f32, kind="ExternalInput")
    bp_d = nc.dram_tensor("bpred", [1, H], f32, kind="ExternalInput")
    bc_d = nc.dram_tensor("bcls", [1, 2], f32, kind="ExternalInput")
    iota_d = nc.dram_tensor("iota", [128, 128], bf16, kind="ExternalInput")
    id_d = nc.dram_tensor("ident", [128, 128], f32, kind="ExternalInput")
    onr_d = nc.dram_tensor("ones_r", [1, 128], f32, kind="ExternalInput")
    onc_d = nc.dram_tensor("ones_c", [128, 1], bf16, kind="ExternalInput")
    out_d = nc.dram_tensor("out", [1, 2], f32, kind="ExternalOutput")

    # ---- internal DRAM ----
    hs_shard = nc.dram_tensor("hs_shard", [NS, H], bf16)
    hs_full = nc.dram_tensor("hs_full", [NPAD, H], bf16)
    sums_in = nc.dram_tensor("sums_in", [4, H], f32)
    sums_out = nc.dram_tensor("sums_out", [4, H], f32)

    hs_lo = hs_full[:HALF, :]
    hs_hi = hs_full[HALF:, :]

    with tile.TileContext(nc) as tc:
        with tc.tile_pool(name="persist", bufs=1) as pp, \
             tc.tile_pool(name="msg", bufs=3) as msgp, \
             tc.tile_pool(name="oh", bufs=3) as ohp, \
             tc.tile_pool(name="work", bufs=4) as wp, \
             tc.tile_pool(name="ps_agg", bufs=3, space="PSUM") as ps_agg, \
             tc.tile_pool(name="ps_hn", bufs=3, space="PSUM") as ps_hn, \
             tc.tile_pool(name="ps_tr", bufs=2, space="PSUM") as ps_tr:

            # ---- persistent SBUF ----
            idxs = pp.tile([128, E_PAD // 16], i16)
            dstl = pp.tile([128, NB], bf16)
            rdeg = pp.tile([128, NT], f32)
            indeg = pp.tile([1, NS], f32)
            wn = pp.tile([128, 2, H], f32)
            wg = pp.tile([128, H], f32)
            wpr = pp.tile([128, 4, H], f32)
            wc = pp.tile([128, 2], f32)
            bn = pp.tile([1, H], f32)
            bg = pp.tile([1, H], f32)
            bp = pp.tile([1, H], f32)
            bc = pp.tile([1, 2], f32)
            iota = pp.tile([128, 128], bf16)
            ident = pp.tile([128, 128], f32)
            onr = pp.tile([1, 128], f32)
            onc = pp.tile([128, 1], bf16)
            hT = pp.tile([128, NS], f32)       # current h, transposed [feat, node]
            sums0 = pp.tile([1, H], f32)       # column sums of h0
            sumsL = pp.tile([128, 3], f32)     # column sums of h1..h3 (feat-major)

            for t_sb, t_d in ((idxs, idxs_d), (dstl, dstl_d), (rdeg, rdeg_d),
                              (indeg, indeg_d), (wn, wn_d), (wg, wg_d),
                              (wpr, wp_d), (wc, wc_d), (bn, bn_d), (bg, bg_d),
                              (bp, bp_d), (bc, bc_d), (iota, iota_d),
                              (ident, id_d), (onr, onr_d), (onc, onc_d)):
                nc.sync.dma_start(t_sb[:], t_d[:])
            nc.vector.memset(sums0[:], 0.0)
            nc.vector.memset(sumsL[:], 0.0)

            # ================= phase 0: h0 = nf @ Wn (bias deferred) ========
            for t in range(NT):
                nft = wp.tile([128, NF], f32, tag="nft")
                nc.sync.dma_start(nft[:], nf_d[t * 128:(t + 1) * 128, :])
                h0p = ps_hn.tile([128, H], f32, space="PSUM", tag="hnp")
                for k in range(2):
                    trp = ps_tr.tile([128, 128], f32, space="PSUM", tag="trp")
                    nc.tensor.transpose(trp[:], nft[:, k * 128:(k + 1) * 128],
                                        ident[:])
                    nfT = wp.tile([128, 128], f32, tag="nfT")
                    nc.vector.tensor_copy(nfT[:], trp[:])
                    nc.tensor.matmul(h0p[:], lhsT=nfT[:], rhs=wn[:, k, :],
                                     start=(k == 0), stop=(k == 1))
                h0b = wp.tile([128, H], bf16, tag="h0b")
                nc.vector.tensor_copy(h0b[:], h0p[:])
                nc.sync.dma_start(hs_shard[t * 128:(t + 1) * 128, :], h0b[:])
                # column sums of h0 (bf16 rounding washed out by the mean)
                m0p = ps_agg.tile([1, H], f32, space="PSUM", tag="aggp")
                nc.tensor.matmul(m0p[:], lhsT=onc[:], rhs=h0b[:],
                                 start=True, stop=True)
                nc.vector.tensor_add(sums0[:], sums0[:], m0p[:])

            if not no_cc:
                nc.gpsimd.collective_compute(
                    "AllGather", OP.bypass, replica_groups=[list(range(P))],
                    ins=[hs_shard[:].opt()], outs=[hs_full[:].opt()])

            # ============ rounds: fusion (r=0) + 3 GCN layers (r=1..3) ======
            qrr = [0]   # round-robin SWDGE queue counter
            NROUNDS = int(os.environ.get("GCN_ROUNDS", "4"))
            for r in range(NROUNDS):
                blk = 0            # global block counter (dstl column)
                lo_done = [0] * NT  # consumed lo blocks per tile
                hi_done = [0] * NT
                for g in groups:
                    g_nlo = sum(nlo[t] for t in g)
                    g_nhi = sum(nhi[t] for t in g)
                    e0 = blk * 128
                    # SWDGE descriptor ring holds 1024 descs; keep each
                    # gather at <= 7 blocks (896 descriptors)
                    GB = 7
                    lo_buf = msgp.tile([128, max(g_nlo, 1), H], bf16, tag="lo")
                    if no_gather and g_nlo:
                        nc.vector.memset(lo_buf[:, :g_nlo, :], 0.0)
                    for off in range(0, g_nlo, GB):
                        if no_gather:
                            break
                        w = min(GB, g_nlo - off)
                        ee = e0 + off * 128
                        nc.gpsimd.dma_gather(
                            out_ap=lo_buf[:, off:off + w, :], in_ap=hs_lo,
                            idxs_ap=idxs[:, ee // 16:(ee + w * 128) // 16],
                            num_idxs=w * 128, num_idxs_reg=w * 128,
                            elem_size=H, queue_num=qrr[0] % 4)
                        qrr[0] += 1
                    e1 = e0 + g_nlo * 128
                    hi_buf = msgp.tile([128, max(g_nhi, 1), H], bf16, tag="hi")
                    if no_gather and g_nhi:
                        nc.vector.memset(hi_buf[:, :g_nhi, :], 0.0)
                    for off in range(0, g_nhi, GB):
                        if no_gather:
                            break
                        w = min(GB, g_nhi - off)
                        ee = e1 + off * 128
                        nc.gpsimd.dma_gather(
                            out_ap=hi_buf[:, off:off + w, :], in_ap=hs_hi,
                            idxs_ap=idxs[:, ee // 16:(ee + w * 128) // 16],
                            num_idxs=w * 128, num_idxs_reg=w * 128,
                            elem_size=H, queue_num=qrr[0] % 4)
                        qrr[0] += 1

                    nb_g = g_nlo + g_nhi

                    # per-tile PSUM accumulation; one-hots built just-in-time
                    # in batches of 8 blocks so tile liveness stays short
                    loc_lo = 0
                    loc_hi = 0
                    for ti, t in enumerate(g):
                        aggp = ps_agg.tile([128, 128], f32, space="PSUM",
                                           tag="aggp")
                        mms = []   # (buf, slot_in_buf, block_in_chunk)
                        for hf in (0, 1):
                            nb_t = (nlo if hf == 0 else nhi)[t]
                            buf = lo_buf if hf == 0 else hi_buf
                            loc0 = loc_lo if hf == 0 else loc_hi
                            gb0 = 0 if hf == 0 else g_nlo
                            for j in range(nb_t):
                                mms.append((buf, loc0 + j, gb0 + loc0 + j))
                            if hf == 0:
                                loc_lo += nb_t
                            else:
                                loc_hi += nb_t
                        for ob in range(0, len(mms), 8):
                            batch = mms[ob:ob + 8]
                            w = len(batch)
                            # blocks in a batch may be non-contiguous in the
                            # chunk (lo->hi jump); build one-hots per
                            # contiguous run within the batch
                            oh = ohp.tile([128, 8, 128], bf16, tag="oh")
                            runs = []
                            for bi, (_, _, babs) in enumerate(batch):
                                if runs and babs == runs[-1][1] + 1:
                                    runs[-1][1] = babs
                                else:
                                    runs.append([babs, babs, bi])
                            if no_oh:
                                nc.vector.memset(oh[:], 0.0)
                            for b0, b1, bi0 in runs:
                                if no_oh:
                                    break
                                rw = b1 - b0 + 1
                                nc.vector.tensor_tensor(
                                    out=oh[:, bi0:bi0 + rw, :],
                                    in0=dstl[:, blk + b0:blk + b1 + 1, None]
                                        .to_broadcast([128, rw, 128]),
                                    in1=iota[:, None, :]
                                        .to_broadcast([128, rw, 128]),
                                    op=OP.is_equal)
                            for bi, (buf, sl, babs) in enumerate(batch):
                                mi = ob + bi
                                last = (mi == len(mms) - 1) and r != 0
                                if no_mm:
                                    if mi == 0 or last:
                                        nc.tensor.matmul(
                                            aggp[:], lhsT=buf[:, sl, :],
                                            rhs=oh[:, bi, :],
                                            start=(mi == 0), stop=last)
                                else:
                                    nc.tensor.matmul(
                                        aggp[:], lhsT=buf[:, sl, :],
                                        rhs=oh[:, bi, :],
                                        start=(mi == 0), stop=last)
                        if r == 0:
                            # fusion: + indeg (X) bn  (rank-1 bias correction,
                            # exact because h0 was stored without its bias)
                            nc.tensor.matmul(aggp[:], lhsT=bn[:],
                                             rhs=indeg[:, t * 128:(t + 1) * 128],
                                             start=(len(mms) == 0), stop=True)
                        hcol = hT[:, t * 128:(t + 1) * 128]
                        if r == 0:
                            # hA = aggT (store for round 1 self-term)
                            nc.vector.tensor_copy(hcol, aggp[:])
                            src_for_hs = hcol
                        else:
                            aself = wp.tile([128, 128], f32, tag="aself")
                            nc.vector.tensor_add(aself[:], aggp[:], hcol)
                            hnp = ps_hn.tile([128, 128], f32, space="PSUM",
                                             tag="hnp")
                            nc.tensor.matmul(hnp[:], lhsT=wg[:], rhs=aself[:],
                                             start=True, stop=False)
                            nc.tensor.matmul(hnp[:], lhsT=bg[:], rhs=onr[:],
                                             start=False, stop=True)
                            # leaky relu: max(x, 0.01x)
                            tmp = wp.tile([128, 128], f32, tag="tmp")
                            nc.vector.tensor_scalar_mul(tmp[:], hnp[:], NEG)
                            nc.vector.tensor_tensor(out=hcol, in0=hnp[:],
                                                    in1=tmp[:], op=OP.max)
                            if t == NT - 1:
                                nc.vector.memset(hT[:, NS_REAL:NS], 0.0)
                            red = wp.tile([128, 1], f32, tag="red")
                            nc.vector.reduce_sum(red[:], hcol, axis=AX)
                            nc.vector.tensor_add(sumsL[:, r - 1:r],
                                                 sumsL[:, r - 1:r], red[:])
                            src_for_hs = hcol
                        if r < 3:
                            # transpose + scale by 1/deg -> bf16 -> hs_shard
                            trp2 = ps_tr.tile([128, 128], f32, space="PSUM",
                                              tag="trp")
                            nc.tensor.transpose(trp2[:], src_for_hs, ident[:])
                            hsb = wp.tile([128, 128], bf16, tag="hsb")
                            nc.vector.tensor_tensor(
                                out=hsb[:], in0=trp2[:],
                                in1=rdeg[:, t:t + 1].to_broadcast([128, 128]),
                                op=OP.mult)
                            nc.sync.dma_start(
                                hs_shard[t * 128:(t + 1) * 128, :], hsb[:])
                    blk += nb_g
                if r < 3 and not no_cc:
                    nc.gpsimd.collective_compute(
                        "AllGather", OP.bypass,
                        replica_groups=[list(range(P))],
                        ins=[hs_shard[:].opt()], outs=[hs_full[:].opt()])

            # ==================== finale =================================
            nc.sync.dma_start(sums_in[0:1, :], sums0[:])
            for rr in range(3):
                nc.sync.dma_start(sums_in[rr + 1:rr + 2, :], sumsL[:, rr:rr + 1])
            nc.gpsimd.collective_compute(
                "AllReduce", OP.add, replica_groups=[list(range(P))],
                ins=[sums_in[:].opt()], outs=[sums_out[:].opt()])
            gsb = wp.tile([4, H], f32, tag="gsb")
            nc.sync.dma_start(gsb[:], sums_out[:])
            gsc = wp.tile([4, H], f32, tag="gsc")
            nc.vector.tensor_scalar_mul(gsc[:], gsb[:], 1.0 / N_REAL)
            nc.vector.tensor_add(gsc[0:1, :], gsc[0:1, :], bn[:])
            gTp = ps_tr.tile([128, 4], f32, space="PSUM", tag="trp")
            nc.tensor.transpose(gTp[:], gsc[:], ident[:4, :4])
            gT = wp.tile([128, 4], f32, tag="gT")
            nc.vector.tensor_copy(gT[:], gTp[:])
            gpp = ps_hn.tile([1, H], f32, space="PSUM", tag="hnp")
            for k in range(4):
                nc.tensor.matmul(gpp[:], lhsT=gT[:, k:k + 1], rhs=wpr[:, k, :],
                                 start=(k == 0), stop=(k == 3))
            gp = wp.tile([1, H], f32, tag="gp")
            nc.vector.tensor_add(gp[:], gpp[:], bp[:])
            gpTp = ps_tr.tile([128, 1], f32, space="PSUM", tag="trp")
            nc.tensor.transpose(gpTp[:], gp[:], ident[:1, :1])
            gpT = wp.tile([128, 1], f32, tag="gpT")
            nc.vector.tensor_copy(gpT[:], gpTp[:])
            lgp = ps_agg.tile([1, 2], f32, space="PSUM", tag="aggp")
            nc.tensor.matmul(lgp[:], lhsT=gpT[:], rhs=wc[:], start=True,
                             stop=True)
            lg = wp.tile([1, 2], f32, tag="lg")
            nc.vector.tensor_add(lg[:], lgp[:], bc[:])
            # softmax over 2 logits
            mx = wp.tile([1, 1], f32, tag="mx")
            nc.vector.reduce_max(mx[:], lg[:], axis=AX)
            ex = wp.tile([1, 2], f32, tag="ex")
            nc.vector.tensor_tensor(out=ex[:], in0=lg[:],
                                    in1=mx[:].to_broadcast([1, 2]),
                                    op=OP.subtract)
            nc.scalar.activation(ex[:], ex[:], mybir.ActivationFunctionType.Exp)
            sm = wp.tile([1, 1], f32, tag="sm")
            nc.vector.reduce_sum(sm[:], ex[:], axis=AX)
            rcp = wp.tile([1, 1], f32, tag="rcp")
            nc.vector.reciprocal(rcp[:], sm[:])
            pr = wp.tile([1, 2], f32, tag="pr")
            nc.vector.tensor_tensor(out=pr[:], in0=ex[:],
                                    in1=rcp[:].to_broadcast([1, 2]),
                                    op=OP.mult)
            nc.sync.dma_start(out_d[:], pr[:])

    nc.compile()
    return nc


TRACE = False          # set by test.py to collect an NTFF profile
LAST_RESULT = None     # BassKernelResults of the last run (for test.py)


def kernel(**inputs):
    global LAST_RESULT
    in_maps, meta = _preprocess(
        inputs["node_feat"], inputs["degree"], inputs["src"], inputs["dst"],
        inputs["Wn"], inputs["bn"], inputs["We"], inputs["be"],
        inputs["Wgcn"], inputs["bgcn"], inputs["Wpred"], inputs["bpred"],
        inputs["Wcls"], inputs["bcls"], inputs["base_data"],
        inputs["edge_feat"])
    nc = _build(meta)
    res = bass_utils.run_bass_kernel_spmd(nc, in_maps, core_ids=list(range(P)),
                                          trace=TRACE)
    LAST_RESULT = res
    return np.asarray(res.results[0]["out"], dtype=np.float32)
